# revision 13
# baseline (speedup 1.0000x reference)
"""Trainium2 Bass kernel for nn_ConvHDC (binary HDC conv encoder + classifier).

Strategy: shard the D=10000 hyperdimension across 8 NeuronCores (pad to 10240
-> 1280 channels/core = 10 tiles of 128 partitions). Everything after conv1 is
depthwise / per-channel, so the whole network runs locally per core; each core
returns its partial [16,10] similarity matrix and the host sums the 8 partials
(the unshard step for the channel-sharded contraction).

Per-core pipeline:
  conv1  : TensorE matmul, one K=50 fp16 matmul per 512-col PSUM chunk.
           Patches are split hi/mid (p = hi + mid/2048, both fp16) and stacked
           along the contraction dim; weights are sign(W1) duplicated with the
           second copy pre-scaled by 1/2048. ~23-bit effective precision.
  BN1    : no stats pass over the conv output at all. mean = w1s.s / N and
           E[x^2] = (w1s^T G w1s)/N with G = patches @ patches^T computed by
           TensorE up front (G via 22 accumulating matmuls over position
           chunks; an appended ones-column gives the row sums s for free).
           All 10 tiles' thresholds are ready before conv1 finishes.
  binarize1: h1 = (x > thr1) in {0,1} on Pool (tensor_scalar is_gt). BN2 is
           invariant to the per-channel affine h->(h+1)/2 provided its eps is
           scaled to EPS/4.
  conv2  : depthwise 3x3/s2 as 9 accumulating TensorE matmuls per group with
           diagonalized per-channel weights (built once for all tiles by
           gpsimd affine_select), bf16 (exact: h1 in {0,1}).
  BN2    : bn_stats/bn_aggr with eps EPS/4 (see above); sign2 on ScalarE
           produces true +-1 h2.
  conv3  : depthwise 6x6 -> 1x1 on DVE (broadcast multiply + reduce), fp32.
  BN3    : bn_stats/bn_aggr, plain EPS; sign3 -> +-1 h3.
  final  : [16,10] partial sims via 10 accumulating bf16 matmuls, scaled by
           1/sqrt(D); host sums the 8 per-core partials.
"""

import sys

if "/opt/trn_rl_repo" not in sys.path:
    sys.path.insert(0, "/opt/trn_rl_repo")

import numpy as np
from numpy.lib.stride_tricks import sliding_window_view

from concourse import bacc, tile, mybir
from concourse import bass as bass_mod

F32 = mybir.dt.float32
F16 = mybir.dt.float16
BF16 = mybir.dt.bfloat16
ALU = mybir.AluOpType
ACTF = mybir.ActivationFunctionType

NCORES = 8
D = 10000
DPAD = 10240
DP = DPAD // NCORES          # 1280 channels per core
DT = DP // 128               # 10 tiles of 128 channels
B = 16
EPS = 1e-5

H1 = 13                      # conv1 output spatial
N1 = B * H1 * H1             # 2704
CH1 = [512, 512, 512, 512, 512, 144]   # conv1 chunk sizes (PSUM banks)
NPCH = 22                    # patchesT chunks of 128 positions (2816 padded)
H2 = 6
N2 = B * H2 * H2             # 576
NB2 = 8                      # images per conv2 group (2 groups of 288 cols)
G2 = N2 // 2                 # 288

_CACHE = {}


def _build_bass():
    """Build + compile the 8-core Bass program. Returns the Bacc object."""
    nc = bacc.Bacc("TRN2", target_bir_lowering=False, debug=False,
                   num_devices=NCORES)

    ptc_d = nc.dram_tensor("ptc", [128, NPCH, 26], F32, kind="ExternalInput").ap()
    w1f_d = nc.dram_tensor("w1f", [50, DP], F32, kind="ExternalInput").ap()
    p16_d = nc.dram_tensor("p16", [50, N1], F16, kind="ExternalInput").ap()
    w2_d = nc.dram_tensor("w2", [128, DT, 9], F32, kind="ExternalInput").ap()
    bn_d = nc.dram_tensor("bn", [128, DT, 6], F32, kind="ExternalInput").ap()
    w3_d = nc.dram_tensor("w3", [128, DT, 36], F32, kind="ExternalInput").ap()
    wct_d = nc.dram_tensor("wct", [128, DT, 10], F32, kind="ExternalInput").ap()
    rsc_d = nc.dram_tensor("rsc", [50, 1], F32, kind="ExternalInput").ap()
    out_d = nc.dram_tensor("sims", [B, 10], F32, kind="ExternalOutput").ap()

    with tile.TileContext(nc) as tc:
        with (
            tc.tile_pool(name="const", bufs=1) as const,
            tc.tile_pool(name="work", bufs=2) as work,
            tc.tile_pool(name="stat", bufs=2) as stat,
            tc.tile_pool(name="psum1", bufs=1, space="PSUM") as psum1,
            tc.tile_pool(name="psum2", bufs=1, space="PSUM") as psum2,
        ):
            # ---------------- input DMAs (order = priority) ----------------
            ptc = const.tile([128, NPCH, 26], F32)
            nc.sync.dma_start(out=ptc[:], in_=ptc_d[:])
            w1f = const.tile([50, DP], F32)
            nc.sync.dma_start(out=w1f[:], in_=w1f_d[:])
            p16 = const.tile([50, N1], F16)
            nc.sync.dma_start(out=p16[:], in_=p16_d[:])
            w2raw = const.tile([128, DT, 9], F32)
            nc.sync.dma_start(out=w2raw[:], in_=w2_d[:])
            bnt = const.tile([128, DT, 6], F32)
            nc.sync.dma_start(out=bnt[:], in_=bn_d[:])
            w3t = const.tile([128, DT, 36], F32)
            nc.sync.dma_start(out=w3t[:], in_=w3_d[:])
            wcraw = const.tile([128, DT, 10], F32)
            nc.sync.dma_start(out=wcraw[:], in_=wct_d[:])

            # ---------------- weight / constant prep ----------------
            # w1 signs: fp32 copy (Gram path) + fp16 hi/mid stacked (conv1)
            w1s32 = const.tile([25, DP], F32)
            nc.scalar.sign(w1s32[:], w1f[0:25, :])
            w1s16r = const.tile([50, DP], F16)
            nc.scalar.sign(w1s16r[:], w1f[:])
            rowscale = const.tile([50, 1], F32)
            nc.sync.dma_start(out=rowscale[:], in_=rsc_d[:])
            w1s16 = const.tile([50, DP], F16)
            nc.vector.tensor_scalar(w1s16[:], w1s16r[:], rowscale[:], None,
                                    ALU.mult)

            w2s = const.tile([128, DT, 9], BF16)
            nc.scalar.sign(w2s[:], w2raw[:])
            wcs = const.tile([128, DT, 10], BF16)
            nc.scalar.sign(wcs[:], wcraw[:])

            epsc = const.tile([128, 1], F32)
            nc.vector.memset(epsc[:], EPS)
            ones25 = const.tile([25, 1], F32)
            nc.vector.memset(ones25[:], 1.0)

            # beMrg[:, t, L] = (beta_L - 0.5) / gamma_L   (gamma > 0 assumed)
            rg = const.tile([128, DT, 3], F32)
            nc.vector.reciprocal(rg[:], bnt[:, :, 0::2])
            beMrg = const.tile([128, DT, 3], F32)
            nc.vector.tensor_scalar(beMrg[:], bnt[:, :, 1::2], -0.5, None,
                                    ALU.add)
            nc.vector.tensor_tensor(beMrg[:], beMrg[:], rg[:], ALU.mult)

            # diagonalized conv2 weights for all tiles (Pool, overlaps head)
            diag9 = const.tile([128, DT, 9, 128], BF16)
            for t in range(DT):
                nc.gpsimd.affine_select(
                    out=diag9[:, t],
                    in_=w2s[:, t, :].unsqueeze(2).broadcast_to([128, 9, 128]),
                    pattern=[[0, 9], [1, 128]], base=0, channel_multiplier=-1,
                    compare_op=ALU.is_equal, fill=0.0)

            # ---------------- Gram path: BN1 stats via TensorE ----------------
            # G_ext = [G | s]: G = P P^T over all 2704 positions, s = row sums
            g_ps = psum2.tile([25, 26], F32, tag="c2_0", name="g_ps")
            for c in range(NPCH):
                nc.tensor.matmul(g_ps[:], lhsT=ptc[:, c, 0:25],
                                 rhs=ptc[:, c, :],
                                 start=(c == 0), stop=(c == NPCH - 1))
            g_sb = const.tile([25, 26], F32)
            nc.scalar.copy(g_sb[:], g_ps[:])

            # Y = G @ w1s (fp32), Z = w1s . Y  => sum_t Z[t,c] = E[x^2]*N
            ych = [512, 512, 256]
            ytags = ["c2_0", "c2_1", "c1_5"]
            ypools = [psum2, psum2, psum1]
            z_sb = const.tile([25, DP], F32)
            yps = []
            off = 0
            for yi, (w, tg, pl) in enumerate(zip(ych, ytags, ypools)):
                yp = pl.tile([25, w], F32, tag=tg, name=f"y_{yi}")
                nc.tensor.matmul(yp[:], lhsT=g_sb[:, 0:25],
                                 rhs=w1s32[:, off:off + w],
                                 start=True, stop=True)
                yps.append((yp, off, w))
                off += w

            # conv1 for tile 0 (PE fills while DVE computes Z)
            p1_t0 = []
            off = 0
            for ci, csz in enumerate(CH1):
                pt = psum1.tile([128, csz], F32, tag=f"c1_{ci}",
                                name=f"p1_0_{ci}")
                nc.tensor.matmul(pt[:], lhsT=w1s16[:, 0:128],
                                 rhs=p16[:, off:off + csz],
                                 start=True, stop=True)
                p1_t0.append(pt)
                off += csz

            for yp, off, w in yps:
                nc.vector.tensor_tensor(z_sb[:, off:off + w], yp[:],
                                        w1s32[:, off:off + w], ALU.mult)

            # per-tile one-column matmuls: Sx and Sx^2 into one PSUM bank
            st_ps = psum2.tile([128, 20], F32, tag="c2_0", name="st_ps")
            for t in range(DT):
                sl = slice(t * 128, (t + 1) * 128)
                nc.tensor.matmul(st_ps[:, 2 * t:2 * t + 1],
                                 lhsT=w1s32[:, sl], rhs=g_sb[:, 25:26],
                                 start=True, stop=True)
                nc.tensor.matmul(st_ps[:, 2 * t + 1:2 * t + 2],
                                 lhsT=z_sb[:, sl], rhs=ones25[:],
                                 start=True, stop=True)
            st_sb = const.tile([128, 20], F32)
            nc.scalar.copy(st_sb[:], st_ps[:])

            # batched thresholds for all tiles: thr1 = mu - sd*beMrg1
            mu = stat.tile([128, 10], F32, tag="mu")
            nc.vector.tensor_scalar(mu[:], st_sb[:, 0::2], 1.0 / N1, None,
                                    ALU.mult)
            ex2 = stat.tile([128, 10], F32, tag="ex2")
            nc.vector.tensor_scalar(ex2[:], st_sb[:, 1::2], 1.0 / N1, None,
                                    ALU.mult)
            mu2 = stat.tile([128, 10], F32, tag="mu2")
            nc.vector.tensor_tensor(mu2[:], mu[:], mu[:], ALU.mult)
            var1 = stat.tile([128, 10], F32, tag="var1")
            nc.vector.tensor_tensor(var1[:], ex2[:], mu2[:], ALU.subtract)
            sd1 = stat.tile([128, 10], F32, tag="sd1")
            nc.scalar.activation(sd1[:], var1[:], ACTF.Sqrt, bias=epsc[:],
                                 scale=1.0)
            sdb = stat.tile([128, 10], F32, tag="sdb")
            nc.vector.tensor_tensor(sdb[:], sd1[:], beMrg[:, :, 0], ALU.mult)
            thr1 = const.tile([128, 10], F32)
            nc.vector.tensor_tensor(thr1[:], mu[:], sdb[:], ALU.subtract)
            bias1 = const.tile([128, 10], F32)
            nc.vector.tensor_tensor(bias1[:], sdb[:], mu[:], ALU.subtract)

            # ---------------- per-tile pipeline ----------------
            h3b_all = const.tile([128, DT, B], BF16)
            p1_cur = p1_t0
            for t in range(DT):
                # binarize1 -> +-1 bf16. Chunks 0-2: ScalarE Sign from PSUM.
                # Chunks 3-5: DVE is_gt ({0,1}) then Pool 2x-1 in SBUF.
                h1b = work.tile([128, N1], BF16, tag="h1b")
                off = 0
                for ci, csz in enumerate(CH1):
                    dst = h1b[:, off:off + csz]
                    if ci < 3:
                        nc.scalar.activation(dst, p1_cur[ci][:], ACTF.Sign,
                                             bias=bias1[:, t:t + 1], scale=1.0)
                    else:
                        nc.vector.tensor_scalar(dst, p1_cur[ci][:],
                                                thr1[:, t:t + 1], None,
                                                ALU.is_gt)
                        nc.gpsimd.tensor_scalar(dst, dst, 2.0, -1.0,
                                                ALU.mult, ALU.add)
                    off += csz

                # conv1 for tile t+1 first on PE: each chunk can start as
                # soon as tile t's compare drains that PSUM bank
                if t + 1 < DT:
                    p1_nxt = []
                    off = 0
                    for ci, csz in enumerate(CH1):
                        pt = psum1.tile([128, csz], F32, tag=f"c1_{ci}",
                                        name=f"p1_{t + 1}_{ci}")
                        nc.tensor.matmul(
                            pt[:], lhsT=w1s16[:, (t + 1) * 128:(t + 2) * 128],
                            rhs=p16[:, off:off + csz], start=True, stop=True)
                        p1_nxt.append(pt)
                        off += csz

                # conv2: diagonalized depthwise 3x3 stride 2
                h1b4 = h1b[:].rearrange("p (b h w) -> p b h w", b=B, h=H1,
                                        w=H1)
                p2 = [psum2.tile([128, G2], F32, tag=f"c2_{g}",
                                 name=f"p2_{t}_{g}") for g in range(2)]
                for k in range(9):
                    kh, kw = k // 3, k % 3
                    for g in range(2):
                        rhs = h1b4[:, g * NB2:(g + 1) * NB2,
                                   kh:kh + 11:2, kw:kw + 11:2]
                        nc.tensor.matmul(p2[g][:], lhsT=diag9[:, t, k, :],
                                         rhs=rhs,
                                         start=(k == 0), stop=(k == 8))
                if t + 1 < DT:
                    p1_cur = p1_nxt

                # BN2 (eps/4 because h1 is {0,1}) + sign2 -> +-1 h2
                st2 = stat.tile([128, 2, 6], F32, tag="st2")
                for g in range(2):
                    nc.vector.bn_stats(st2[:, g, :], p2[g][:])
                mv2 = stat.tile([128, 2], F32, tag="mv2")
                nc.vector.bn_aggr(mv2[:], st2[:])
                sq2 = stat.tile([128, 1], F32, tag="sq2")
                nc.scalar.activation(sq2[:], mv2[:, 1:2], ACTF.Sqrt,
                                     bias=epsc[:], scale=1.0)
                bias2 = stat.tile([128, 1], F32, tag="bias2")
                nc.vector.tensor_scalar(bias2[:], sq2[:], beMrg[:, t, 1:2],
                                        mv2[:, 0:1], ALU.mult, ALU.subtract)
                h2b = work.tile([128, N2], BF16, tag="h2b")
                for g in range(2):
                    nc.scalar.activation(h2b[:, g * G2:(g + 1) * G2],
                                         p2[g][:], ACTF.Sign, bias=bias2[:],
                                         scale=1.0)

                # conv3: depthwise 6x6 -> 1 (Pool; SBUF-only engine)
                tmp3 = work.tile([128, B, 36], F32, tag="tmp3")
                h2b3 = h2b[:].rearrange("p (b s) -> p b s", b=B, s=36)
                w3b = w3t[:, t, :].unsqueeze(1).broadcast_to([128, B, 36])
                nc.gpsimd.tensor_tensor(tmp3[:], h2b3, w3b, ALU.mult)
                h3pre = work.tile([128, B], F32, tag="h3pre")
                nc.vector.tensor_reduce(h3pre[:], tmp3[:],
                                        mybir.AxisListType.X, ALU.add)

                st3 = stat.tile([128, 6], F32, tag="st3")
                nc.vector.bn_stats(st3[:], h3pre[:])
                mv3 = stat.tile([128, 2], F32, tag="mv3")
                nc.vector.bn_aggr(mv3[:], st3[:])
                sq3 = stat.tile([128, 1], F32, tag="sq3")
                nc.scalar.activation(sq3[:], mv3[:, 1:2], ACTF.Sqrt,
                                     bias=epsc[:], scale=1.0)
                bias3 = stat.tile([128, 1], F32, tag="bias3")
                nc.vector.tensor_scalar(bias3[:], sq3[:], beMrg[:, t, 2:3],
                                        mv3[:, 0:1], ALU.mult, ALU.subtract)
                nc.scalar.activation(h3b_all[:, t, :], h3pre[:], ACTF.Sign,
                                     bias=bias3[:], scale=1.0)

            # ---------------- classifier (partial sims; host sums) ----------
            psims = psum2.tile([B, 10], F32, tag="c2_0", name="psims")
            for t in range(DT):
                nc.tensor.matmul(psims[:], lhsT=h3b_all[:, t, :],
                                 rhs=wcs[:, t, :],
                                 start=(t == 0), stop=(t == DT - 1))
            sims_sb = stat.tile([B, 10], F32, tag="sims_sb")
            nc.scalar.mul(sims_sb[:], psims[:], 1.0 / np.sqrt(np.float32(D)))
            nc.sync.dma_start(out=out_d[:], in_=sims_sb[:])

    nc.compile()
    return nc


def get_nc():
    if "nc" not in _CACHE:
        _CACHE["nc"] = _build_bass()
    return _CACHE["nc"]


def prep_inputs(x, W1, b1, g1, be1, W2, b2, g2, be2, W3, b3, g3, be3, Wc):
    """Host-side layout/sharding prep (layout only, no model math).

    Conv biases b1/b2/b3 are dropped: training-mode BN is invariant to a
    per-channel additive constant before normalization.
    """
    f = np.float32

    xp = np.zeros((B, 30, 30), f)
    xp[:, 1:29, 1:29] = np.asarray(x, f)[:, 0]
    win = sliding_window_view(xp, (5, 5), axis=(1, 2))[:, ::2, ::2]
    patches = np.ascontiguousarray(
        win.transpose(3, 4, 0, 1, 2).reshape(25, N1)).astype(f)

    # fp16 hi/mid split: p ~= hi + mid/2048 (stacked on contraction dim)
    p_hi = patches.astype(np.float16)
    p_mid = ((patches - p_hi.astype(f)) * 2048.0).astype(np.float16)
    p16 = np.concatenate([p_hi, p_mid], axis=0)          # [50, N1] f16

    # patchesT chunks + ones column for the Gram path
    npad = NPCH * 128                                     # 2816
    ptc = np.zeros((npad, 26), f)
    ptc[:N1, :25] = patches.T
    ptc[:N1, 25] = 1.0
    ptc = np.ascontiguousarray(
        ptc.reshape(NPCH, 128, 26).transpose(1, 0, 2))    # [128, 22, 26]

    def padrows(a, width, fill=0.0):
        out = np.full((DPAD, width), fill, f)
        out[:D] = np.asarray(a, f).reshape(D, width)
        return out

    w1p = padrows(W1, 25)                                 # [DPAD, 25]
    w2p = padrows(W2, 9)
    w3p = padrows(W3, 36)
    wcp = padrows(np.asarray(Wc, f).T, 10)
    bn = np.zeros((DPAD, 6), f)
    bn[:, 0::2] = 1.0                                     # pad gamma=1
    bn[:D, 0] = np.asarray(g1, f)
    bn[:D, 1] = np.asarray(be1, f)
    bn[:D, 2] = np.asarray(g2, f)
    bn[:D, 3] = np.asarray(be2, f)
    bn[:D, 4] = np.asarray(g3, f)
    bn[:D, 5] = np.asarray(be3, f)

    def tiled(a, width):
        # [DP, width] -> [128, DT, width]
        return np.ascontiguousarray(
            a.reshape(DT, 128, width).transpose(1, 0, 2))

    rsc = np.concatenate([np.full((25, 1), 1.0, f),
                          np.full((25, 1), 1.0 / 2048.0, f)])

    in_maps = []
    for c in range(NCORES):
        sl = slice(c * DP, (c + 1) * DP)
        w1c = np.ascontiguousarray(w1p[sl].T)             # [25, DP]
        in_maps.append({
            "ptc": ptc,
            "w1f": np.concatenate([w1c, w1c], axis=0),    # [50, DP]
            "p16": p16,
            "w2": tiled(w2p[sl], 9),
            "bn": tiled(bn[sl], 6),
            "w3": tiled(w3p[sl], 36),
            "wct": tiled(wcp[sl], 10),
            "rsc": rsc,
        })
    return in_maps


def kernel(**inputs) -> np.ndarray:
    from concourse.bass_utils import run_bass_kernel_spmd
    nc = get_nc()
    in_maps = prep_inputs(**inputs)
    res = run_bass_kernel_spmd(nc, in_maps, list(range(NCORES)))
    acc = np.zeros((B, 10), np.float32)
    for c in range(NCORES):
        acc += np.asarray(res.results[c]["sims"], np.float32)
    return acc
